# revision 62
# baseline (speedup 1.0000x reference)
"""Trainium2 Bass kernel for an AttentionBlock (GroupNorm -> 1x1-conv QKV ->
softmax attention -> 1x1-conv projection -> residual).

Sharding: 8 cores = (batch b in 0..3) x (half of the h*w=4096 query positions).
Each core gets the full x[b] row (fp16) for keys/values plus its query slice,
and produces the [64, 2048] output slice.

v2 design (vs the v1 baseline):
  - Entire q/k projection pipeline collapsed into one tiny per-block matmul:
    the k-bias is a per-query constant in the scores and cancels in softmax;
    what remains is  S^T = x16^T u  with  u = A o (Wk^T Wq (A o x_q)) + A o vb,
    vb = Wk^T Wq B + Wk^T bq.  Host ships Wq^T Wk; the two GroupNorm folds
    (A o .) ride existing per-partition-scalar DVE ops.  ST's stationary
    operand is the raw fp16 x tile (duplicated on partitions 64..127 for
    K=64 row packing).
  - exp split across ACT (native Exp -> fp8e4) and DVE (Schraudolph:
    u8 = round(log2(e)*s + 7.65), saturating-to-0 on negatives (HW verified),
    bitcast fp8e4).  Both paths fold a shift of -ln(64) so every fp8 value
    stays under 240: the PE decodes fp8e4 as IEEE-style e4m3 whose
    exponent-15 codes (>240) are inf/NaN, NOT e4m3fn's 256..448.
  - PV runs in fp8 with perf_mode=DoubleRow: one matmul per PAIR of j-tiles
    (K=256).  Weights are scaled by 16 (|16w| <= 99 < 240) and carry an extra
    16.0-column so the PV accumulator also emits the softmax denominators;
    both fp8 scales cancel exactly in the softmax division.  Software-
    pipelined with lag 2 and three rotating 2-bank PSUM chunk buffers.
  - GroupNorm stats from quarter-tiles split across DVE (sums) and ACT
    (Square+accum), rstd via bit-trick Newton; fp16 x shipped from host.
"""

import numpy as np
import ml_dtypes

import concourse.bacc as bacc
import concourse.bass as bass
import concourse.tile as tile
from concourse import mybir
from concourse.bass_utils import run_bass_kernel_spmd

F32 = mybir.dt.float32
F16 = mybir.dt.float16
F8 = mybir.dt.float8e4
U8 = mybir.dt.uint8
AF = mybir.ActivationFunctionType
ALU = mybir.AluOpType
PM = mybir.MatmulPerfMode

C = 64          # channels
N = 4096        # h*w
NQ = 2048       # query columns per core
NB = 4          # query blocks of 512
QB = 512        # query block width
JT = 128        # j tile width
NPAIR = 16      # j-tile pairs per block (32 j tiles)
NGROUPS = 8
EPS = 1e-5
GSIZE = C // NGROUPS * N  # elements per (batch, group) = 32768

# The PE decodes fp8e4 as IEEE-style e4m3: exponent-15 codes (values > 240,
# u8 code >= 0x78) are inf/NaN, NOT e4m3fn's 256..448 range (HW-verified:
# ACT's convert clamps to 0x78 = +inf and the PE NaN-poisons PSUM on code
# >= 120).  So keep ALL fp8 values <= 240.
# exp scale: e = exp(s/8 - ln 64) -> ACT values <= 205, DVE codes <= 118.
# weight scale: wg = 16*w -> |16w| <= 99.  The ones-column is 16.0 so the
# softmax division cancels WSC exactly; the e-scale cancels by construction.
ESC = 64.0
WSC = 16.0
SHIFT = -float(np.log(ESC))          # -4.158883
EXP_A8 = float(np.log2(np.e))        # 8 * 0.125 * log2(e)
EXP_B8 = 8.0 * (SHIFT * float(np.log2(np.e)) + 7.0) - 0.35  # = 7.65

# packed constants layout (columns of the [64, CP_COLS] fp32 "cpack" input)
CP_W2T = 0       # (Wp@Wv)^T fp32 (for folded epilogue bias)
CP_G = 64        # group indicator G [64, 8]
CP_BTOT = 72     # Wp@bv + bp
CP_GAMMA = 73
CP_BETA = 74
CP_GT = 75       # G^T [8, 64] on partitions 0..7
CP_WQK = 139     # (Wq^T Wk) fp32, duplicated -> [64, 128] (for vb matmul)
CP_COLS = 267

DBG_B = 1
DBG_PAIRS = (12, 13)

# per-block pair -> engine assignment: 'A' = ACT, 'D' = DVE
PAIRS_BY_BLOCK = [
    "ADAADADAADADADAA",  # 10 A, 6 D
    "ADADADAADADADADA",  # 9 A, 7 D
    "ADAADADAADADADAA",
    "ADADADAADADADADA",
]


def build_bass(reps=1):
    nc = bacc.Bacc("TRN2", target_bir_lowering=False, debug=False, num_devices=8)
    _emit(nc, reps)
    nc.compile()
    return nc


def _emit_dbg(nc):
    _emit(nc, reps=1, dbg=True)


def _emit(nc, reps=1, dbg=False):
    xb16_d = nc.dram_tensor("xb16", [C, N], F16, kind="ExternalInput")
    xq_d = nc.dram_tensor("xq", [C, NQ], F32, kind="ExternalInput")
    cpack = nc.dram_tensor("cpack", [C, CP_COLS], F32, kind="ExternalInput")
    cpk2_d = nc.dram_tensor("cpk2", [2 * C, 1], F32, kind="ExternalInput")
    wqk16_d = nc.dram_tensor("wqk16", [C, 2 * C], F16, kind="ExternalInput")
    w2tb_d = nc.dram_tensor("w2tb", [C, C], F16, kind="ExternalInput")
    out_d = nc.dram_tensor("out", [C, NQ], F32, kind="ExternalOutput")
    if dbg:
        du2_d = nc.dram_tensor("d_u2", [2 * C, QB], F16, kind="ExternalOutput")
        dwg_d = nc.dram_tensor("d_wg", [JT, 2 * NPAIR, 80], U8,
                               kind="ExternalOutput")
        de8a_d = nc.dram_tensor("d_e8a", [2 * C, 2 * QB], U8,
                                kind="ExternalOutput")
        de8d_d = nc.dram_tensor("d_e8d", [2 * C, 2 * QB], U8,
                                kind="ExternalOutput")
        de8all_d = nc.dram_tensor("d_e8all", [2 * C, NPAIR * 2 * QB], U8,
                                  kind="ExternalOutput")
        dpv_d = nc.dram_tensor("d_pv", [C + 1, QB], F32, kind="ExternalOutput")

    NQ4 = N // 4

    with tile.TileContext(nc) as tc:
        with (
            tc.tile_pool(name="consts", bufs=1) as consts,
            tc.tile_pool(name="big", bufs=1) as big,
            tc.tile_pool(name="epool", bufs=8) as epool,
            tc.tile_pool(name="small", bufs=2) as small,
            tc.tile_pool(name="ps_a", bufs=1, space="PSUM") as ps_a,
            tc.tile_pool(name="ps_b", bufs=1, space="PSUM") as ps_b,
            tc.tile_pool(name="ps_c", bufs=1, space="PSUM") as ps_c,
            tc.tile_pool(name="ps_pv", bufs=1, space="PSUM") as ps_pv,
        ):
          chunk_pools = [ps_a, ps_b, ps_c]
          chunk_tags = ["a1", "b1", "c1"]
          for _rep in range(reps):
            # dummy exp triggers the one ACT table load while DMAs fly
            warm = consts.tile([1, 1], F32, tag="warm")
            nc.vector.memset(warm, 1.0)
            nc.scalar.activation(out=warm, in_=warm, func=AF.Exp, bias=0.0,
                                 scale=1.0)
            sh_t = consts.tile([2 * C, 1], F32, tag="sh")
            nc.vector.memset(sh_t, SHIFT)
            ones_r = consts.tile([1, C], mybir.dt.bfloat16, tag="onesr")
            nc.vector.memset(ones_r, 1.0)

            # ---- inputs ----
            # xd: [128, 4096] fp16; rows 0-63 = x16 in quarters (stats start
            # early), rows 64-127 = dup (ST row packing).
            xd = big.tile([2 * C, N], F16, tag="xd")
            cp = consts.tile([C, CP_COLS], F32, tag="cp")
            cpk2 = consts.tile([2 * C, 1], F32, tag="cpk2")
            wqk16 = consts.tile([C, 2 * C], F16, tag="wqk16")
            w2tb = consts.tile([C, C], F16, tag="w2tb")
            xq_s = big.tile([C, NQ], F32, tag="xq")
            for q in range(2):
                nc.gpsimd.dma_start(out=xd[0:C, q * NQ4:(q + 1) * NQ4],
                                    in_=xb16_d[:, q * NQ4:(q + 1) * NQ4])
            for q in range(2, 4):
                nc.sync.dma_start(out=xd[0:C, q * NQ4:(q + 1) * NQ4],
                                  in_=xb16_d[:, q * NQ4:(q + 1) * NQ4])
            nc.sync.dma_start(out=cp, in_=cpack[:, :])
            nc.sync.dma_start(out=xd[C:2 * C, 0:N // 2],
                              in_=xb16_d[:, 0:N // 2])
            nc.sync.dma_start(out=xd[C:2 * C, N // 2:], in_=xb16_d[:, N // 2:])
            nc.gpsimd.dma_start(out=cpk2, in_=cpk2_d[:, :])
            nc.gpsimd.dma_start(out=wqk16, in_=wqk16_d[:, :])
            nc.gpsimd.dma_start(out=w2tb, in_=w2tb_d[:, :])
            nc.sync.dma_start(out=xq_s, in_=xq_d[:, :])

            # ---- GroupNorm stats on quarters: sums on DVE, sumsq on ACT ----
            s12h = big.tile([C, 2, 4], F32, tag="s12h")
            scr_a = big.tile([C, NQ4], F16, tag="scra")
            for q in range(4):
                nc.vector.reduce_sum(out=s12h[:, 0, q:q + 1],
                                     in_=xd[0:C, q * NQ4:(q + 1) * NQ4],
                                     axis=mybir.AxisListType.X)
                nc.scalar.activation(out=scr_a,
                                     in_=xd[0:C, q * NQ4:(q + 1) * NQ4],
                                     func=AF.Square,
                                     accum_out=s12h[:, 1, q:q + 1])
            s12 = big.tile([C, 2], F32, tag="s12")
            nc.vector.tensor_reduce(out=s12, in_=s12h,
                                    axis=mybir.AxisListType.X,
                                    op=ALU.add)
            gstat = ps_pv.tile([NGROUPS, 2], F32, tag="pv")
            nc.tensor.matmul(out=gstat, lhsT=cp[:, CP_G:CP_G + NGROUPS],
                             rhs=s12, start=True, stop=True)

            # group mean / var -> rstd via bit-trick + 2 Newton iters (DVE)
            tmv = big.tile([NGROUPS, 2], F32, tag="tmv")
            nc.vector.tensor_scalar_mul(out=tmv, in0=gstat, scalar1=1.0 / GSIZE)
            var = big.tile([NGROUPS, 1], F32, tag="var")
            nc.vector.tensor_mul(out=var, in0=tmv[:, 0:1], in1=tmv[:, 0:1])
            nc.vector.tensor_sub(out=var, in0=tmv[:, 1:2], in1=var)
            tgrp = big.tile([NGROUPS, 2], F32, tag="tgrp")
            veps = big.tile([NGROUPS, 1], F32, tag="veps")
            vh = big.tile([NGROUPS, 1], F32, tag="vh")
            nc.vector.tensor_scalar_add(out=veps, in0=var, scalar1=EPS)
            nc.vector.tensor_scalar_mul(out=vh, in0=veps, scalar1=0.5)
            magic = consts.tile([NGROUPS, 1], mybir.dt.int32, tag="magic")
            nc.vector.memset(magic, 0x5F3759DF)
            c15 = consts.tile([NGROUPS, 1], F32, tag="c15")
            nc.vector.memset(c15, 1.5)
            y_i = big.tile([NGROUPS, 1], mybir.dt.int32, tag="yi")
            nc.vector.tensor_scalar(
                out=y_i, in0=veps.bitcast(mybir.dt.int32), scalar1=1,
                scalar2=None, op0=ALU.arith_shift_right,
            )
            nc.vector.tensor_sub(out=y_i, in0=magic, in1=y_i)
            y_f = y_i.bitcast(F32)
            t_n = big.tile([NGROUPS, 1], F32, tag="tn")
            for _it in range(2):
                nc.vector.tensor_mul(out=t_n, in0=y_f, in1=y_f)
                nc.vector.tensor_mul(out=t_n, in0=t_n, in1=vh)
                nc.vector.scalar_tensor_tensor(
                    out=t_n, in0=t_n, scalar=-1.0, in1=c15,
                    op0=ALU.mult, op1=ALU.add,
                )
                nc.vector.tensor_mul(out=y_f, in0=y_f, in1=t_n)
            nc.vector.tensor_copy(out=tgrp[:, 0:1], in_=y_f)
            nc.vector.tensor_copy(out=tgrp[:, 1:2], in_=tmv[:, 0:1])

            # expand [8,2] -> [64,2] per-channel on the PE
            gexp_ps = ps_pv.tile([C, 2], F32, tag="pv")
            nc.tensor.matmul(out=gexp_ps, lhsT=cp[0:NGROUPS, CP_GT:CP_GT + C],
                             rhs=tgrp, start=True, stop=True)
            a_s = big.tile([C, 1], F32, tag="a")
            b_s = big.tile([C, 1], F32, tag="b")
            nc.vector.tensor_mul(out=a_s, in0=gexp_ps[:, 0:1],
                                 in1=cp[:, CP_GAMMA:CP_GAMMA + 1])
            nc.vector.tensor_mul(out=b_s, in0=gexp_ps[:, 1:2], in1=a_s)
            nc.vector.tensor_sub(out=b_s, in0=cp[:, CP_BETA:CP_BETA + 1],
                                 in1=b_s)
            # a duplicated on both partition halves (for the u scale)
            a_s2 = big.tile([2 * C, 1], F32, tag="a2")
            nc.sync.dma_start(out=a_s2[0:C, :], in_=a_s)
            nc.gpsimd.dma_start(out=a_s2[C:2 * C, :], in_=a_s)

            # vb = (Wk^T Wq) B + Wk^T bq  (second term host-shipped in cpk2)
            vb_ps = ps_pv.tile([2 * C, 1], F32, tag="pv")
            nc.tensor.matmul(out=vb_ps, lhsT=cp[:, CP_WQK:CP_WQK + 2 * C],
                             rhs=b_s, start=True, stop=True)
            vb2 = big.tile([2 * C, 1], F32, tag="vb2")
            nc.vector.tensor_add(out=vb2, in0=vb_ps, in1=cpk2)

            # folded weights / epilogue bias
            w2t2 = big.tile([C, C], F16, tag="w2t2")
            nc.vector.tensor_scalar_mul(out=w2t2, in0=w2tb, scalar1=a_s)
            b2b_ps = ps_pv.tile([C, 1], F32, tag="pv")
            nc.tensor.matmul(out=b2b_ps, lhsT=cp[:, CP_W2T:CP_W2T + C],
                             rhs=b_s, start=True, stop=True)
            btot2 = big.tile([C, 1], F32, tag="btot2")
            nc.vector.tensor_add(out=btot2, in0=b2b_ps,
                                 in1=cp[:, CP_BTOT:CP_BTOT + 1])

            # xqa = A o x_q (query half pre-scale; 4x DVE mode)
            xqa = big.tile([C, NQ], F16, tag="xqa")
            nc.vector.tensor_scalar_mul(out=xqa, in0=xd[0:C, 0:NQ],
                                        scalar1=a_s)
            # xqb = xq + btot2 (epilogue residual+bias, precombined)
            xqb = big.tile([C, NQ], F32, tag="xqb")
            nc.vector.tensor_scalar_add(out=xqb, in0=xq_s, scalar1=btot2)

            # ---- u projection for one block; called lazily inline ----
            u2t = [None] * NB

            def emit_u(b):
                # side-chain buffer: keep the ST chunk rings free
                up = ps_pv.tile([2 * C, QB], F32, tag="rbb")
                nc.tensor.matmul(out=up, lhsT=wqk16,
                                 rhs=xqa[:, b * QB:(b + 1) * QB],
                                 start=True, stop=True)
                u2_b = big.tile([2 * C, QB], F16, tag=f"u2{b}")
                nc.vector.tensor_scalar(out=u2_b, in0=up, scalar1=vb2,
                                        scalar2=a_s2, op0=ALU.add,
                                        op1=ALU.mult)
                u2t[b] = u2_b

            # ---- PV weights: wg8[:, jt, 0:64] = 64*(W2' x)^T fp8,
            #      col 64 = 64.0 (denominator column) ----
            wg8 = big.tile([JT, 2 * NPAIR, 80], F8, tag="wg8")
            nc.vector.memset(wg8[:, :, C:C + 1], WSC)

            def emit_wgroup(g):
                wp_ps = ps_pv.tile([JT, 8, C], F32, tag="rbb")
                for t in range(8):
                    jt = 8 * g + t
                    nc.tensor.matmul(
                        out=wp_ps[:, t, :],
                        lhsT=xd[0:C, jt * JT:(jt + 1) * JT], rhs=w2t2,
                        start=True, stop=True,
                    )
                if g != 2:
                    nc.scalar.activation(out=wg8[:, 8 * g:8 * (g + 1), 0:C],
                                         in_=wp_ps, func=AF.Copy, bias=0.0,
                                         scale=float(WSC))
                else:
                    nc.vector.tensor_scalar_mul(
                        out=wg8[:, 8 * g:8 * (g + 1), 0:C], in0=wp_ps,
                        scalar1=float(WSC))

            emit_u(0)
            emit_wgroup(0)
            emit_wgroup(1)

            # ---- attention ----
            for b in range(NB):
                LAG = 4 if b < NB - 1 else 1
                pvtag = "pv"
                pv_ps = ps_pv.tile([C + 1, QB], F32, tag=pvtag)
                inflight = []  # e8 tiles not yet consumed by PV
                for p in range(NPAIR):
                    eng = PAIRS_BY_BLOCK[b][p]
                    ci = (b * NPAIR + p) % 3
                    st_ps = chunk_pools[ci].tile([2 * C, 2, QB], F32,
                                                 tag=chunk_tags[ci])
                    for half in range(2):
                        jt = 2 * p + half
                        ro = 0 if (b == 0 and p < 2) else C * half
                        nc.tensor.matmul(
                            out=st_ps[:, half, :],
                            lhsT=xd[ro:ro + C, jt * JT:(jt + 1) * JT],
                            rhs=u2t[b][ro:ro + C, :],
                            start=True, stop=True,
                        )
                    e8 = epool.tile([2 * C, 2, QB], U8, tag="e")
                    if eng == "A":
                        nc.scalar.activation(
                            out=e8.bitcast(F8), in_=st_ps, func=AF.Exp,
                            bias=sh_t, scale=0.125,
                        )
                    else:
                        nc.vector.tensor_scalar(
                            out=e8, in0=st_ps, scalar1=EXP_A8, scalar2=EXP_B8,
                            op0=ALU.mult, op1=ALU.add,
                        )
                    if dbg and b == DBG_B and p in DBG_PAIRS:
                        nc.sync.dma_start(
                            out=(de8a_d if p == DBG_PAIRS[0] else de8d_d)[:, :],
                            in_=e8.bitcast(U8))
                    if dbg and b == DBG_B:
                        nc.sync.dma_start(
                            out=de8all_d[:, p * 2 * QB:(p + 1) * 2 * QB],
                            in_=e8.bitcast(U8))
                    inflight.append((e8, p))
                    if len(inflight) > LAG:
                        pe8, pp = inflight.pop(0)
                        nc.tensor.matmul(
                            out=pv_ps, lhsT=wg8[:, 2 * pp:2 * pp + 2, 0:C + 1],
                            rhs=pe8.bitcast(F8), start=(pp == 0),
                            stop=False, perf_mode=PM.DoubleRow,
                            skip_group_check=True,
                        )
                    # interleave w-groups / remaining u-projections into
                    # block 0's pipeline-fill trough
                    if b == 0 and p in (1, 3):
                        emit_wgroup(p // 2 + 2)
                    if b == 0 and p in (2, 5, 8):
                        emit_u(p // 3 + 1)
                for k, (pe8, pp) in enumerate(inflight):
                    nc.tensor.matmul(
                        out=pv_ps, lhsT=wg8[:, 2 * pp:2 * pp + 2, 0:C + 1],
                        rhs=pe8.bitcast(F8), start=(pp == 0),
                        stop=(k == len(inflight) - 1),
                        perf_mode=PM.DoubleRow, skip_group_check=True,
                    )

                # epilogue: recip reads the PSUM denominator row directly,
                # overlapping the pv PSUM->SBUF copy
                pv_sb = small.tile([C + 1, QB], F32, tag="pvsb")
                if b == NB - 1:
                    nc.scalar.activation(out=pv_sb, in_=pv_ps, func=AF.Copy,
                                         bias=0.0, scale=1.0)
                else:
                    nc.vector.tensor_copy(out=pv_sb, in_=pv_ps)
                rb_s0 = small.tile([1, QB], mybir.dt.bfloat16, tag="rb0")
                with nc.allow_low_precision(reason="1/D broadcast in bf16"):
                    nc.vector.reciprocal(out=rb_s0, in_=pv_ps[C:C + 1, :])
                if dbg and b == DBG_B:
                    nc.sync.dma_start(out=du2_d[:, :], in_=u2t[DBG_B])
                    nc.sync.dma_start(out=dwg_d[:, :, :], in_=wg8.bitcast(U8))
                    nc.sync.dma_start(out=dpv_d[:, :], in_=pv_sb)
                o_s = small.tile([C, QB], F32, tag="o")
                o2 = small.tile([C, QB], F32, tag="o2")
                rbb_ps = ps_pv.tile([C, QB], F32, tag="rbb")
                nc.tensor.matmul(out=rbb_ps, lhsT=ones_r, rhs=rb_s0,
                                 start=True, stop=True)
                nc.vector.tensor_mul(out=o_s, in0=pv_sb[0:C, :],
                                     in1=rbb_ps)
                nc.gpsimd.tensor_add(out=o2, in0=o_s,
                                     in1=xqb[:, b * QB:(b + 1) * QB])
                nc.sync.dma_start(out=out_d[:, b * QB:(b + 1) * QB], in_=o2)


_NC = None


def _get_nc():
    global _NC
    if _NC is None:
        _NC = build_bass()
    return _NC


def make_in_maps(x, gamma, beta, Wq, bq, Wk, bk, Wv, bv, Wp, bp):
    x = np.asarray(x, np.float32)
    b, c, h, w = x.shape
    n = h * w
    xf = np.ascontiguousarray(x.reshape(b, c, n))
    Wq = np.asarray(Wq, np.float32)
    Wk = np.asarray(Wk, np.float32)
    W2 = np.asarray(Wp, np.float32) @ np.asarray(Wv, np.float32)
    btot = np.asarray(Wp, np.float32) @ np.asarray(bv, np.float32) + \
        np.asarray(bp, np.float32)
    WQK = Wq.T @ Wk                      # [c, c]
    vbc = Wk.T @ np.asarray(bq, np.float32)
    cpk = np.zeros((C, CP_COLS), np.float32)
    cpk[:, CP_W2T:CP_W2T + C] = W2.T
    cpk[np.arange(C), CP_G + np.arange(C) // (C // NGROUPS)] = 1.0
    cpk[:, CP_BTOT] = btot
    cpk[:, CP_GAMMA] = np.asarray(gamma, np.float32)
    cpk[:, CP_BETA] = np.asarray(beta, np.float32)
    cpk[0:NGROUPS, CP_GT:CP_GT + C] = cpk[:, CP_G:CP_G + NGROUPS].T
    cpk[:, CP_WQK:CP_WQK + C] = WQK
    cpk[:, CP_WQK + C:CP_WQK + 2 * C] = WQK
    common = {
        "cpack": cpk,
        "cpk2": np.ascontiguousarray(
            np.concatenate([vbc, vbc])[:, None].astype(np.float32)),
        "wqk16": np.ascontiguousarray(
            np.concatenate([WQK, WQK], axis=1).astype(np.float16)),
        "w2tb": np.ascontiguousarray(W2.T.astype(np.float16)),
    }
    in_maps = []
    for core in range(8):
        bi, hi = divmod(core, 2)
        m = dict(common)
        # arrange xb16 so the query half comes FIRST; j-tile order follows
        # this permuted layout, which softmax doesn't care about.
        qs = xf[bi][:, hi * NQ:(hi + 1) * NQ]
        other = xf[bi][:, (1 - hi) * NQ:(2 - hi) * NQ]
        m["xb16"] = np.ascontiguousarray(
            np.concatenate([qs, other], axis=1).astype(np.float16))
        m["xq"] = np.ascontiguousarray(qs)
        in_maps.append(m)
    return in_maps


def assemble_out(results, b=4, c=64, h=64, w=64):
    n = h * w
    out = np.empty((b, c, n), np.float32)
    for core in range(8):
        bi, hi = divmod(core, 2)
        out[bi][:, hi * NQ:(hi + 1) * NQ] = results[core]["out"]
    return out.reshape(b, c, h, w)


def kernel(x, gamma, beta, Wq, bq, Wk, bk, Wv, bv, Wp, bp):
    nc = _get_nc()
    in_maps = make_in_maps(x, gamma, beta, Wq, bq, Wk, bk, Wv, bv, Wp, bp)
    res = run_bass_kernel_spmd(nc, in_maps, core_ids=list(range(8)))
    return assemble_out(res.results)


# revision 63
# speedup vs baseline: 1.0079x; 1.0079x over previous
"""Trainium2 Bass kernel for an AttentionBlock (GroupNorm -> 1x1-conv QKV ->
softmax attention -> 1x1-conv projection -> residual).

Sharding: 8 cores = (batch b in 0..3) x (half of the h*w=4096 query positions).
Each core gets the full x[b] row (fp16) for keys/values plus its query slice,
and produces the [64, 2048] output slice.

v2 design (vs the v1 baseline):
  - Entire q/k projection pipeline collapsed into one tiny per-block matmul:
    the k-bias is a per-query constant in the scores and cancels in softmax;
    what remains is  S^T = x16^T u  with  u = A o (Wk^T Wq (A o x_q)) + A o vb,
    vb = Wk^T Wq B + Wk^T bq.  Host ships Wq^T Wk; the two GroupNorm folds
    (A o .) ride existing per-partition-scalar DVE ops.  ST's stationary
    operand is the raw fp16 x tile (duplicated on partitions 64..127 for
    K=64 row packing).
  - exp split across ACT (native Exp -> fp8e4) and DVE (Schraudolph:
    u8 = round(log2(e)*s + 7.65), saturating-to-0 on negatives (HW verified),
    bitcast fp8e4).  Both paths fold a shift of -ln(64) so every fp8 value
    stays under 240: the PE decodes fp8e4 as IEEE-style e4m3 whose
    exponent-15 codes (>240) are inf/NaN, NOT e4m3fn's 256..448.
  - PV runs in fp8 with perf_mode=DoubleRow: one matmul per PAIR of j-tiles
    (K=256).  Weights are scaled by 16 (|16w| <= 99 < 240) and carry an extra
    16.0-column so the PV accumulator also emits the softmax denominators;
    both fp8 scales cancel exactly in the softmax division.  Software-
    pipelined with lag 2 and three rotating 2-bank PSUM chunk buffers.
  - GroupNorm stats from quarter-tiles split across DVE (sums) and ACT
    (Square+accum), rstd via bit-trick Newton; fp16 x shipped from host.
"""

import numpy as np
import ml_dtypes

import concourse.bacc as bacc
import concourse.bass as bass
import concourse.tile as tile
from concourse import mybir
from concourse.bass_utils import run_bass_kernel_spmd

F32 = mybir.dt.float32
F16 = mybir.dt.float16
F8 = mybir.dt.float8e4
U8 = mybir.dt.uint8
AF = mybir.ActivationFunctionType
ALU = mybir.AluOpType
PM = mybir.MatmulPerfMode

C = 64          # channels
N = 4096        # h*w
NQ = 2048       # query columns per core
NB = 4          # query blocks of 512
QB = 512        # query block width
JT = 128        # j tile width
NPAIR = 16      # j-tile pairs per block (32 j tiles)
NGROUPS = 8
EPS = 1e-5
GSIZE = C // NGROUPS * N  # elements per (batch, group) = 32768

# The PE decodes fp8e4 as IEEE-style e4m3: exponent-15 codes (values > 240,
# u8 code >= 0x78) are inf/NaN, NOT e4m3fn's 256..448 range (HW-verified:
# ACT's convert clamps to 0x78 = +inf and the PE NaN-poisons PSUM on code
# >= 120).  So keep ALL fp8 values <= 240.
# exp scale: e = exp(s/8 - ln 64) -> ACT values <= 205, DVE codes <= 118.
# weight scale: wg = 16*w -> |16w| <= 99.  The ones-column is 16.0 so the
# softmax division cancels WSC exactly; the e-scale cancels by construction.
ESC = 64.0
WSC = 16.0
SHIFT = -float(np.log(ESC))          # -4.158883
EXP_A8 = float(np.log2(np.e))        # 8 * 0.125 * log2(e)
EXP_B8 = 8.0 * (SHIFT * float(np.log2(np.e)) + 7.0) - 0.35  # = 7.65

# packed constants layout (columns of the [64, CP_COLS] fp32 "cpack" input)
CP_W2T = 0       # (Wp@Wv)^T fp32 (for folded epilogue bias)
CP_G = 64        # group indicator G [64, 8]
CP_BTOT = 72     # Wp@bv + bp
CP_GAMMA = 73
CP_BETA = 74
CP_GT = 75       # G^T [8, 64] on partitions 0..7
CP_WQK = 139     # (Wq^T Wk) fp32, duplicated -> [64, 128] (for vb matmul)
CP_COLS = 267

DBG_B = 1
DBG_PAIRS = (12, 13)

# per-block pair -> engine assignment: 'A' = ACT, 'D' = DVE
PAIRS_BY_BLOCK = [
    "ADAADADAADADADAA",  # 10 A, 6 D
    "ADADADAADADADADA",  # 9 A, 7 D
    "ADAADADAADADADAA",
    "ADADADAADADADADA",
]


def build_bass(reps=1):
    nc = bacc.Bacc("TRN2", target_bir_lowering=False, debug=False, num_devices=8)
    _emit(nc, reps)
    nc.compile()
    return nc


def _emit_dbg(nc):
    _emit(nc, reps=1, dbg=True)


def _emit(nc, reps=1, dbg=False):
    xb16_d = nc.dram_tensor("xb16", [C, N], F16, kind="ExternalInput")
    xq_d = nc.dram_tensor("xq", [C, NQ], F32, kind="ExternalInput")
    cpack = nc.dram_tensor("cpack", [C, CP_COLS], F32, kind="ExternalInput")
    cpk2_d = nc.dram_tensor("cpk2", [2 * C, 1], F32, kind="ExternalInput")
    wqk16_d = nc.dram_tensor("wqk16", [C, 2 * C], F16, kind="ExternalInput")
    w2tb_d = nc.dram_tensor("w2tb", [C, C], F16, kind="ExternalInput")
    out_d = nc.dram_tensor("out", [C, NQ], F32, kind="ExternalOutput")
    if dbg:
        du2_d = nc.dram_tensor("d_u2", [2 * C, QB], F16, kind="ExternalOutput")
        dwg_d = nc.dram_tensor("d_wg", [JT, 2 * NPAIR, 80], U8,
                               kind="ExternalOutput")
        de8a_d = nc.dram_tensor("d_e8a", [2 * C, 2 * QB], U8,
                                kind="ExternalOutput")
        de8d_d = nc.dram_tensor("d_e8d", [2 * C, 2 * QB], U8,
                                kind="ExternalOutput")
        de8all_d = nc.dram_tensor("d_e8all", [2 * C, NPAIR * 2 * QB], U8,
                                  kind="ExternalOutput")
        dpv_d = nc.dram_tensor("d_pv", [C + 1, QB], F32, kind="ExternalOutput")

    NQ4 = N // 4

    with tile.TileContext(nc) as tc:
        with (
            tc.tile_pool(name="consts", bufs=1) as consts,
            tc.tile_pool(name="big", bufs=1) as big,
            tc.tile_pool(name="epool", bufs=12) as epool,
            tc.tile_pool(name="small", bufs=3) as small,
            tc.tile_pool(name="ps_a", bufs=1, space="PSUM") as ps_a,
            tc.tile_pool(name="ps_b", bufs=1, space="PSUM") as ps_b,
            tc.tile_pool(name="ps_c", bufs=1, space="PSUM") as ps_c,
            tc.tile_pool(name="ps_pv", bufs=1, space="PSUM") as ps_pv,
        ):
          chunk_pools = [ps_a, ps_b, ps_c]
          chunk_tags = ["a1", "b1", "c1"]
          for _rep in range(reps):
            # dummy exp triggers the one ACT table load while DMAs fly
            warm = consts.tile([1, 1], F32, tag="warm")
            nc.vector.memset(warm, 1.0)
            nc.scalar.activation(out=warm, in_=warm, func=AF.Exp, bias=0.0,
                                 scale=1.0)
            sh_t = consts.tile([2 * C, 1], F32, tag="sh")
            nc.vector.memset(sh_t, SHIFT)
            ones_r = consts.tile([1, C], mybir.dt.bfloat16, tag="onesr")
            nc.vector.memset(ones_r, 1.0)

            # ---- inputs ----
            # xd: [128, 4096] fp16; rows 0-63 = x16 in quarters (stats start
            # early), rows 64-127 = dup (ST row packing).
            xd = big.tile([2 * C, N], F16, tag="xd")
            cp = consts.tile([C, CP_COLS], F32, tag="cp")
            cpk2 = consts.tile([2 * C, 1], F32, tag="cpk2")
            wqk16 = consts.tile([C, 2 * C], F16, tag="wqk16")
            w2tb = consts.tile([C, C], F16, tag="w2tb")
            xq_s = big.tile([C, NQ], F32, tag="xq")
            for q in range(2):
                nc.gpsimd.dma_start(out=xd[0:C, q * NQ4:(q + 1) * NQ4],
                                    in_=xb16_d[:, q * NQ4:(q + 1) * NQ4])
            for q in range(2, 4):
                nc.sync.dma_start(out=xd[0:C, q * NQ4:(q + 1) * NQ4],
                                  in_=xb16_d[:, q * NQ4:(q + 1) * NQ4])
            nc.sync.dma_start(out=cp, in_=cpack[:, :])
            nc.sync.dma_start(out=xd[C:2 * C, 0:N // 2],
                              in_=xb16_d[:, 0:N // 2])
            nc.sync.dma_start(out=xd[C:2 * C, N // 2:], in_=xb16_d[:, N // 2:])
            nc.gpsimd.dma_start(out=cpk2, in_=cpk2_d[:, :])
            nc.gpsimd.dma_start(out=wqk16, in_=wqk16_d[:, :])
            nc.gpsimd.dma_start(out=w2tb, in_=w2tb_d[:, :])
            nc.sync.dma_start(out=xq_s, in_=xq_d[:, :])

            # ---- GroupNorm stats on quarters: sums on DVE, sumsq on ACT ----
            s12h = big.tile([C, 2, 4], F32, tag="s12h")
            scr_a = big.tile([C, NQ4], F16, tag="scra")
            for q in range(4):
                nc.vector.reduce_sum(out=s12h[:, 0, q:q + 1],
                                     in_=xd[0:C, q * NQ4:(q + 1) * NQ4],
                                     axis=mybir.AxisListType.X)
                nc.scalar.activation(out=scr_a,
                                     in_=xd[0:C, q * NQ4:(q + 1) * NQ4],
                                     func=AF.Square,
                                     accum_out=s12h[:, 1, q:q + 1])
            s12 = big.tile([C, 2], F32, tag="s12")
            nc.vector.tensor_reduce(out=s12, in_=s12h,
                                    axis=mybir.AxisListType.X,
                                    op=ALU.add)
            gstat = ps_pv.tile([NGROUPS, 2], F32, tag="pv")
            nc.tensor.matmul(out=gstat, lhsT=cp[:, CP_G:CP_G + NGROUPS],
                             rhs=s12, start=True, stop=True)

            # group mean / var -> rstd via bit-trick + 2 Newton iters (DVE)
            tmv = big.tile([NGROUPS, 2], F32, tag="tmv")
            nc.vector.tensor_scalar_mul(out=tmv, in0=gstat, scalar1=1.0 / GSIZE)
            var = big.tile([NGROUPS, 1], F32, tag="var")
            nc.vector.tensor_mul(out=var, in0=tmv[:, 0:1], in1=tmv[:, 0:1])
            nc.vector.tensor_sub(out=var, in0=tmv[:, 1:2], in1=var)
            tgrp = big.tile([NGROUPS, 2], F32, tag="tgrp")
            veps = big.tile([NGROUPS, 1], F32, tag="veps")
            vh = big.tile([NGROUPS, 1], F32, tag="vh")
            nc.vector.tensor_scalar_add(out=veps, in0=var, scalar1=EPS)
            nc.vector.tensor_scalar_mul(out=vh, in0=veps, scalar1=0.5)
            magic = consts.tile([NGROUPS, 1], mybir.dt.int32, tag="magic")
            nc.vector.memset(magic, 0x5F3759DF)
            c15 = consts.tile([NGROUPS, 1], F32, tag="c15")
            nc.vector.memset(c15, 1.5)
            y_i = big.tile([NGROUPS, 1], mybir.dt.int32, tag="yi")
            nc.vector.tensor_scalar(
                out=y_i, in0=veps.bitcast(mybir.dt.int32), scalar1=1,
                scalar2=None, op0=ALU.arith_shift_right,
            )
            nc.vector.tensor_sub(out=y_i, in0=magic, in1=y_i)
            y_f = y_i.bitcast(F32)
            t_n = big.tile([NGROUPS, 1], F32, tag="tn")
            for _it in range(2):
                nc.vector.tensor_mul(out=t_n, in0=y_f, in1=y_f)
                nc.vector.tensor_mul(out=t_n, in0=t_n, in1=vh)
                nc.vector.scalar_tensor_tensor(
                    out=t_n, in0=t_n, scalar=-1.0, in1=c15,
                    op0=ALU.mult, op1=ALU.add,
                )
                nc.vector.tensor_mul(out=y_f, in0=y_f, in1=t_n)
            nc.vector.tensor_copy(out=tgrp[:, 0:1], in_=y_f)
            nc.vector.tensor_copy(out=tgrp[:, 1:2], in_=tmv[:, 0:1])

            # expand [8,2] -> [64,2] per-channel on the PE
            gexp_ps = ps_pv.tile([C, 2], F32, tag="pv")
            nc.tensor.matmul(out=gexp_ps, lhsT=cp[0:NGROUPS, CP_GT:CP_GT + C],
                             rhs=tgrp, start=True, stop=True)
            a_s = big.tile([C, 1], F32, tag="a")
            b_s = big.tile([C, 1], F32, tag="b")
            nc.vector.tensor_mul(out=a_s, in0=gexp_ps[:, 0:1],
                                 in1=cp[:, CP_GAMMA:CP_GAMMA + 1])
            nc.vector.tensor_mul(out=b_s, in0=gexp_ps[:, 1:2], in1=a_s)
            nc.vector.tensor_sub(out=b_s, in0=cp[:, CP_BETA:CP_BETA + 1],
                                 in1=b_s)
            # a duplicated on both partition halves (for the u scale)
            a_s2 = big.tile([2 * C, 1], F32, tag="a2")
            nc.sync.dma_start(out=a_s2[0:C, :], in_=a_s)
            nc.gpsimd.dma_start(out=a_s2[C:2 * C, :], in_=a_s)

            # vb = (Wk^T Wq) B + Wk^T bq  (second term host-shipped in cpk2)
            vb_ps = ps_pv.tile([2 * C, 1], F32, tag="pv")
            nc.tensor.matmul(out=vb_ps, lhsT=cp[:, CP_WQK:CP_WQK + 2 * C],
                             rhs=b_s, start=True, stop=True)
            vb2 = big.tile([2 * C, 1], F32, tag="vb2")
            nc.vector.tensor_add(out=vb2, in0=vb_ps, in1=cpk2)

            # folded weights / epilogue bias
            w2t2 = big.tile([C, C], F16, tag="w2t2")
            nc.vector.tensor_scalar_mul(out=w2t2, in0=w2tb, scalar1=a_s)
            b2b_ps = ps_pv.tile([C, 1], F32, tag="pv")
            nc.tensor.matmul(out=b2b_ps, lhsT=cp[:, CP_W2T:CP_W2T + C],
                             rhs=b_s, start=True, stop=True)
            btot2 = big.tile([C, 1], F32, tag="btot2")
            nc.vector.tensor_add(out=btot2, in0=b2b_ps,
                                 in1=cp[:, CP_BTOT:CP_BTOT + 1])

            # xqa = A o x_q (query half pre-scale; 4x DVE mode)
            xqa = big.tile([C, NQ], F16, tag="xqa")
            nc.vector.tensor_scalar_mul(out=xqa, in0=xd[0:C, 0:NQ],
                                        scalar1=a_s)
            # xqb = xq + btot2 (epilogue residual+bias, precombined)
            xqb = big.tile([C, NQ], F32, tag="xqb")
            nc.vector.tensor_scalar_add(out=xqb, in0=xq_s, scalar1=btot2)

            # ---- u projection for one block; called lazily inline ----
            u2t = [None] * NB

            def emit_u(b):
                # side-chain buffer: keep the ST chunk rings free
                up = ps_pv.tile([2 * C, QB], F32, tag="rbb")
                nc.tensor.matmul(out=up, lhsT=wqk16,
                                 rhs=xqa[:, b * QB:(b + 1) * QB],
                                 start=True, stop=True)
                u2_b = big.tile([2 * C, QB], F16, tag=f"u2{b}")
                nc.vector.tensor_scalar(out=u2_b, in0=up, scalar1=vb2,
                                        scalar2=a_s2, op0=ALU.add,
                                        op1=ALU.mult)
                u2t[b] = u2_b

            # ---- PV weights: wg8[:, jt, 0:64] = 64*(W2' x)^T fp8,
            #      col 64 = 64.0 (denominator column) ----
            wg8 = big.tile([JT, 2 * NPAIR, 80], F8, tag="wg8")
            nc.vector.memset(wg8[:, :, C:C + 1], WSC)

            def emit_wgroup(g):
                wp_ps = ps_pv.tile([JT, 8, C], F32, tag="rbb")
                for t in range(8):
                    jt = 8 * g + t
                    nc.tensor.matmul(
                        out=wp_ps[:, t, :],
                        lhsT=xd[0:C, jt * JT:(jt + 1) * JT], rhs=w2t2,
                        start=True, stop=True,
                    )
                if g != 2:
                    nc.scalar.activation(out=wg8[:, 8 * g:8 * (g + 1), 0:C],
                                         in_=wp_ps, func=AF.Copy, bias=0.0,
                                         scale=float(WSC))
                else:
                    nc.vector.tensor_scalar_mul(
                        out=wg8[:, 8 * g:8 * (g + 1), 0:C], in0=wp_ps,
                        scalar1=float(WSC))

            emit_u(0)
            emit_wgroup(0)
            emit_wgroup(1)

            # ---- attention ----
            for b in range(NB):
                LAG = 4 if b < NB - 1 else 1
                pvtag = "pv"
                pv_ps = ps_pv.tile([C + 1, QB], F32, tag=pvtag)
                inflight = []  # e8 tiles not yet consumed by PV
                for p in range(NPAIR):
                    eng = PAIRS_BY_BLOCK[b][p]
                    ci = (b * NPAIR + p) % 3
                    st_ps = chunk_pools[ci].tile([2 * C, 2, QB], F32,
                                                 tag=chunk_tags[ci])
                    for half in range(2):
                        jt = 2 * p + half
                        ro = 0 if (b == 0 and p < 2) else C * half
                        nc.tensor.matmul(
                            out=st_ps[:, half, :],
                            lhsT=xd[ro:ro + C, jt * JT:(jt + 1) * JT],
                            rhs=u2t[b][ro:ro + C, :],
                            start=True, stop=True,
                        )
                    e8 = epool.tile([2 * C, 2, QB], U8, tag="e")
                    if eng == "A":
                        nc.scalar.activation(
                            out=e8.bitcast(F8), in_=st_ps, func=AF.Exp,
                            bias=sh_t, scale=0.125,
                        )
                    else:
                        nc.vector.tensor_scalar(
                            out=e8, in0=st_ps, scalar1=EXP_A8, scalar2=EXP_B8,
                            op0=ALU.mult, op1=ALU.add,
                        )
                    if dbg and b == DBG_B and p in DBG_PAIRS:
                        nc.sync.dma_start(
                            out=(de8a_d if p == DBG_PAIRS[0] else de8d_d)[:, :],
                            in_=e8.bitcast(U8))
                    if dbg and b == DBG_B:
                        nc.sync.dma_start(
                            out=de8all_d[:, p * 2 * QB:(p + 1) * 2 * QB],
                            in_=e8.bitcast(U8))
                    inflight.append((e8, p))
                    if len(inflight) > LAG:
                        pe8, pp = inflight.pop(0)
                        nc.tensor.matmul(
                            out=pv_ps, lhsT=wg8[:, 2 * pp:2 * pp + 2, 0:C + 1],
                            rhs=pe8.bitcast(F8), start=(pp == 0),
                            stop=False, perf_mode=PM.DoubleRow,
                            skip_group_check=True,
                        )
                    # interleave w-groups / remaining u-projections into
                    # block 0's pipeline-fill trough
                    if b == 0 and p in (1, 3):
                        emit_wgroup(p // 2 + 2)
                    if b == 0 and p in (2, 5, 8):
                        emit_u(p // 3 + 1)
                for k, (pe8, pp) in enumerate(inflight):
                    nc.tensor.matmul(
                        out=pv_ps, lhsT=wg8[:, 2 * pp:2 * pp + 2, 0:C + 1],
                        rhs=pe8.bitcast(F8), start=(pp == 0),
                        stop=(k == len(inflight) - 1),
                        perf_mode=PM.DoubleRow, skip_group_check=True,
                    )

                # epilogue: recip reads the PSUM denominator row directly,
                # overlapping the pv PSUM->SBUF copy
                pv_sb = small.tile([C + 1, QB], F32, tag="pvsb")
                if b == NB - 1:
                    nc.scalar.activation(out=pv_sb, in_=pv_ps, func=AF.Copy,
                                         bias=0.0, scale=1.0)
                else:
                    nc.vector.tensor_copy(out=pv_sb, in_=pv_ps)
                rb_s0 = small.tile([1, QB], mybir.dt.bfloat16, tag="rb0")
                with nc.allow_low_precision(reason="1/D broadcast in bf16"):
                    nc.vector.reciprocal(out=rb_s0, in_=pv_ps[C:C + 1, :])
                if dbg and b == DBG_B:
                    nc.sync.dma_start(out=du2_d[:, :], in_=u2t[DBG_B])
                    nc.sync.dma_start(out=dwg_d[:, :, :], in_=wg8.bitcast(U8))
                    nc.sync.dma_start(out=dpv_d[:, :], in_=pv_sb)
                o_s = small.tile([C, QB], F32, tag="o")
                o2 = small.tile([C, QB], F32, tag="o2")
                rbb_ps = ps_pv.tile([C, QB], F32, tag="rbb")
                nc.tensor.matmul(out=rbb_ps, lhsT=ones_r, rhs=rb_s0,
                                 start=True, stop=True)
                nc.vector.tensor_mul(out=o_s, in0=pv_sb[0:C, :],
                                     in1=rbb_ps)
                nc.gpsimd.tensor_add(out=o2, in0=o_s,
                                     in1=xqb[:, b * QB:(b + 1) * QB])
                nc.sync.dma_start(out=out_d[:, b * QB:(b + 1) * QB], in_=o2)


_NC = None


def _get_nc():
    global _NC
    if _NC is None:
        _NC = build_bass()
    return _NC


def make_in_maps(x, gamma, beta, Wq, bq, Wk, bk, Wv, bv, Wp, bp):
    x = np.asarray(x, np.float32)
    b, c, h, w = x.shape
    n = h * w
    xf = np.ascontiguousarray(x.reshape(b, c, n))
    Wq = np.asarray(Wq, np.float32)
    Wk = np.asarray(Wk, np.float32)
    W2 = np.asarray(Wp, np.float32) @ np.asarray(Wv, np.float32)
    btot = np.asarray(Wp, np.float32) @ np.asarray(bv, np.float32) + \
        np.asarray(bp, np.float32)
    WQK = Wq.T @ Wk                      # [c, c]
    vbc = Wk.T @ np.asarray(bq, np.float32)
    cpk = np.zeros((C, CP_COLS), np.float32)
    cpk[:, CP_W2T:CP_W2T + C] = W2.T
    cpk[np.arange(C), CP_G + np.arange(C) // (C // NGROUPS)] = 1.0
    cpk[:, CP_BTOT] = btot
    cpk[:, CP_GAMMA] = np.asarray(gamma, np.float32)
    cpk[:, CP_BETA] = np.asarray(beta, np.float32)
    cpk[0:NGROUPS, CP_GT:CP_GT + C] = cpk[:, CP_G:CP_G + NGROUPS].T
    cpk[:, CP_WQK:CP_WQK + C] = WQK
    cpk[:, CP_WQK + C:CP_WQK + 2 * C] = WQK
    common = {
        "cpack": cpk,
        "cpk2": np.ascontiguousarray(
            np.concatenate([vbc, vbc])[:, None].astype(np.float32)),
        "wqk16": np.ascontiguousarray(
            np.concatenate([WQK, WQK], axis=1).astype(np.float16)),
        "w2tb": np.ascontiguousarray(W2.T.astype(np.float16)),
    }
    in_maps = []
    for core in range(8):
        bi, hi = divmod(core, 2)
        m = dict(common)
        # arrange xb16 so the query half comes FIRST; j-tile order follows
        # this permuted layout, which softmax doesn't care about.
        qs = xf[bi][:, hi * NQ:(hi + 1) * NQ]
        other = xf[bi][:, (1 - hi) * NQ:(2 - hi) * NQ]
        m["xb16"] = np.ascontiguousarray(
            np.concatenate([qs, other], axis=1).astype(np.float16))
        m["xq"] = np.ascontiguousarray(qs)
        in_maps.append(m)
    return in_maps


def assemble_out(results, b=4, c=64, h=64, w=64):
    n = h * w
    out = np.empty((b, c, n), np.float32)
    for core in range(8):
        bi, hi = divmod(core, 2)
        out[bi][:, hi * NQ:(hi + 1) * NQ] = results[core]["out"]
    return out.reshape(b, c, h, w)


def kernel(x, gamma, beta, Wq, bq, Wk, bk, Wv, bv, Wp, bp):
    nc = _get_nc()
    in_maps = make_in_maps(x, gamma, beta, Wq, bq, Wk, bk, Wv, bv, Wp, bp)
    res = run_bass_kernel_spmd(nc, in_maps, core_ids=list(range(8)))
    return assemble_out(res.results)
